# revision 1
# baseline (speedup 1.0000x reference)
"""Causal self-attention on 8 Trainium2 NeuronCores.

Full inputs in, full output out. Sharding: core c -> (batch b = c//2,
head-group hg = c%2 covering 8 of 16 heads). Each core computes QKV
projections for its head slice, causal flash-attention in a transposed
layout (S^T = keys x queries, so softmax denominators come from a ones
column appended to V and no on-device transposes are needed), and a
partial output projection over its 512 feature columns. The host sums
the two partials per batch and adds the bias.

All host-side reshapes/transposes (x^T, weight slices) are numpy; the
device consumes them directly. Matmul operands are float16 (full PE
rate, fp32 PSUM accumulation; ~7e-4 end-to-end relative error). The
two heads of a feature block issue S^T matmuls on disjoint PE row
groups (partitions 0-63 / 64-127) so they run concurrently, and
diagonal blocks only compute the causally-live trapezoid of columns.
"""
import sys

if "/opt/trn_rl_repo" not in sys.path:
    sys.path.insert(0, "/opt/trn_rl_repo")

import numpy as np

import concourse.bass as bass
import concourse.tile as tile
from concourse import bacc, mybir
from concourse.bass_utils import run_bass_kernel_spmd

F32 = mybir.dt.float32
F16 = mybir.dt.float16
AF = mybir.ActivationFunctionType

B, T, C = 4, 2048, 1024
H, D = 16, 64
N_CORES = 8
HPC = 8            # heads per core
FPC = HPC * D      # feats per core = 512
NEG = -30000.0
QB = 512           # query block
NQB = T // QB      # 4
NKK = T // 128     # 16 key chunks
NCC = C // 128     # 8 contraction chunks
NFB = FPC // 128   # 4 feature blocks (head pairs)

_cached = {}


def _build_program():
    nc = bacc.Bacc("TRN2", target_bir_lowering=False, debug=False,
                   num_devices=N_CORES)

    xT_d = nc.dram_tensor("xT", [C, T], F16, kind="ExternalInput").ap()
    wqT_d = nc.dram_tensor("wqT", [C, FPC], F16, kind="ExternalInput").ap()
    wkT_d = nc.dram_tensor("wkT", [C, FPC], F16, kind="ExternalInput").ap()
    wvT_d = nc.dram_tensor("wvT", [C, FPC], F16, kind="ExternalInput").ap()
    wpT_d = nc.dram_tensor("wpT", [FPC, C], F16, kind="ExternalInput").ap()
    ident_d = nc.dram_tensor("ident", [128, 128], F16, kind="ExternalInput").ap()
    tri_d = nc.dram_tensor("tri", [128, 128], F16, kind="ExternalInput").ap()
    ones_d = nc.dram_tensor("ones", [1, 64], F16, kind="ExternalInput").ap()
    out_d = nc.dram_tensor("out", [T, C], F32, kind="ExternalOutput").ap()

    with tile.TileContext(nc) as tc:
        with tc.tile_pool(name="persist", bufs=1) as persist:
            qt_sb = persist.tile([128, NFB, T], F16, tag="qt")
            kt_sb = persist.tile([128, NFB, T], F16, tag="kt")
            v_sb = persist.tile([128, NKK, HPC, D + 1], F16, tag="v")
            ident = persist.tile([128, 128], F16, tag="ident")
            tri = persist.tile([128, 128], F16, tag="tri")
            ones_col = persist.tile([1, 64], F16, tag="ones")
            nc.sync.dma_start(ident, ident_d)
            nc.sync.dma_start(tri, tri_d)
            nc.sync.dma_start(ones_col, ones_d)
            # ones column of V' (PV matmul then emits softmax denominators)
            nc.gpsimd.memset(v_sb[:, :, :, D:D + 1], 1.0)

            # ---------------- phase 1: QKV projections ----------------
            with tc.tile_pool(name="wqkv", bufs=1) as wqkv, \
                 tc.tile_pool(name="xt", bufs=2) as xtp, \
                 tc.tile_pool(name="ps1", bufs=4, space="PSUM") as ps1:
                wq_sb = wqkv.tile([128, NCC, FPC], F16, tag="wq")
                wk_sb = wqkv.tile([128, NCC, FPC], F16, tag="wk")
                wv_sb = wqkv.tile([128, NCC, FPC], F16, tag="wv")
                wq_r = wqT_d.rearrange("(c p) f -> p c f", p=128)
                wk_r = wkT_d.rearrange("(c p) f -> p c f", p=128)
                wv_r = wvT_d.rearrange("(c p) f -> p c f", p=128)
                xT_r = xT_d.rearrange("(c p) t -> p c t", p=128)

                # first token block's x lands before the weight bulk so the
                # first matmuls can start early
                xt0 = xtp.tile([128, NCC, QB], F16, tag="xt", name="xt0")
                for cc in range(NCC):
                    nc.sync.dma_start(xt0[:, cc, :], xT_r[:, cc, 0:QB])
                for cc in range(NCC):
                    nc.sync.dma_start(wq_sb[:, cc, :], wq_r[:, cc, :])
                    nc.sync.dma_start(wk_sb[:, cc, :], wk_r[:, cc, :])
                    nc.sync.dma_start(wv_sb[:, cc, :], wv_r[:, cc, :])

                # warm the PE clock gate while input DMAs stream in
                warm = ps1.tile([128, 512], F32, tag="ps1", name="warm")
                for i in range(60):
                    nc.tensor.matmul(warm[:, 0:128], ident, ident,
                                     start=True, stop=True,
                                     skip_group_check=True)

                for tb in range(NQB):
                    if tb == 0:
                        xt = xt0
                    else:
                        xt = xtp.tile([128, NCC, QB], F16, tag="xt",
                                      name="xt")
                        for cc in range(NCC):
                            nc.sync.dma_start(
                                xt[:, cc, :],
                                xT_r[:, cc, tb * QB:(tb + 1) * QB])
                    for fb in range(NFB):
                        qps = ps1.tile([128, QB], F32, tag="ps1", name="qps")
                        for cc in range(NCC):
                            nc.tensor.matmul(
                                qps, wq_sb[:, cc, fb * 128:(fb + 1) * 128],
                                xt[:, cc, :],
                                start=(cc == 0), stop=(cc == NCC - 1))
                        nc.vector.tensor_copy(
                            qt_sb[:, fb, tb * QB:(tb + 1) * QB], qps)
                        kps = ps1.tile([128, QB], F32, tag="ps1", name="kps")
                        for cc in range(NCC):
                            nc.tensor.matmul(
                                kps, wk_sb[:, cc, fb * 128:(fb + 1) * 128],
                                xt[:, cc, :],
                                start=(cc == 0), stop=(cc == NCC - 1))
                        nc.vector.tensor_copy(
                            kt_sb[:, fb, tb * QB:(tb + 1) * QB], kps)
                    for tt in range(4):
                        vps = ps1.tile([128, FPC], F32, tag="ps1", name="vps")
                        for cc in range(NCC):
                            nc.tensor.matmul(
                                vps, xt[:, cc, tt * 128:(tt + 1) * 128],
                                wv_sb[:, cc, :],
                                start=(cc == 0), stop=(cc == NCC - 1))
                        nc.vector.tensor_copy(
                            v_sb[:, tb * 4 + tt, :, 0:D],
                            vps.rearrange("p (h d) -> p h d", h=HPC))

            # ------------- phase 2: attention + projection -------------
            with tc.tile_pool(name="const2", bufs=1) as const2, \
                 tc.tile_pool(name="pt", bufs=4) as ptp, \
                 tc.tile_pool(name="yt", bufs=2) as ytp, \
                 tc.tile_pool(name="rcp", bufs=2) as rcpp, \
                 tc.tile_pool(name="sums", bufs=2) as sumsp, \
                 tc.tile_pool(name="outsb", bufs=3) as outp, \
                 tc.tile_pool(name="st", bufs=2, space="PSUM") as stp, \
                 tc.tile_pool(name="pv", bufs=1, space="PSUM") as pvp, \
                 tc.tile_pool(name="bc", bufs=1, space="PSUM") as bcp, \
                 tc.tile_pool(name="prj", bufs=1, space="PSUM") as prjp:
                wp_sb = const2.tile([128, NFB, C], F16, tag="wp")
                nc.sync.dma_start(wp_sb, wpT_d.rearrange("(c p) f -> p c f", p=128))

                for qb in range(NQB):
                    yt = ytp.tile([128, NFB, QB], F16, tag="yt", name="yt")
                    for fb in range(NFB):
                        pv = [pvp.tile([65, QB], F32, tag=f"pv{h2}",
                                       name=f"pv{h2}")
                              for h2 in range(2)]
                        nkk = 4 * qb + 4
                        for kk in range(nkk):
                            dl = kk - 4 * qb
                            j0 = 128 * dl if dl >= 0 else 0
                            st = stp.tile([128, 2, QB], F32, tag="st",
                                          name="st")
                            for h2 in range(2):
                                p0, p1 = 64 * h2, 64 * h2 + 64
                                nc.tensor.matmul(
                                    st[:, h2, j0:QB],
                                    kt_sb[p0:p1, fb, kk * 128:(kk + 1) * 128],
                                    qt_sb[p0:p1, fb, qb * QB + j0:(qb + 1) * QB],
                                    start=True, stop=True,
                                    skip_group_check=True)
                            ptile = ptp.tile([128, 2, QB], F16, tag="pt",
                                             name="ptile")
                            nc.scalar.activation(
                                ptile[:, :, j0:QB], st[:, :, j0:QB], AF.Exp)
                            if dl >= 0:
                                # zero the causally-dead triangle of the
                                # diagonal band on the vector engine (both
                                # heads in one strided op; the 0-step middle
                                # dim re-reads the same mask tile)
                                band = ptile[:, :, j0:j0 + 128]
                                nc.vector.tensor_mul(
                                    band, band,
                                    bass.AP(tri.tensor, tri.offset,
                                            [tri.ap[0], [0, 2], tri.ap[1]]))
                            for h2 in range(2):
                                h = 2 * fb + h2
                                nc.tensor.matmul(
                                    pv[h2][:, j0:QB], v_sb[:, kk, h, :],
                                    ptile[:, h2, j0:QB],
                                    start=(kk == 0), stop=(kk == nkk - 1),
                                    skip_group_check=True)

                        for h2 in range(2):
                            p0, p1 = 64 * h2, 64 * h2 + 64
                            sums = sumsp.tile([1, QB], F16, tag="sums",
                                              name="sums")
                            nc.vector.tensor_copy(sums, pv[h2][D:D + 1, :])
                            bc = bcp.tile([64, QB], F32, tag="bc", name="bc")
                            nc.tensor.matmul(bc, ones_col, sums,
                                             start=True, stop=True)
                            rcp = rcpp.tile([64, QB], F32, tag="rcp",
                                            name="rcp")
                            nc.vector.reciprocal_approx_fast(out=rcp, in_=bc)
                            nc.vector.tensor_mul(yt[p0:p1, fb, :],
                                                 pv[h2][0:D, :], rcp)
                    for tt in range(4):
                        osb = outp.tile([128, C], F32, tag="osb", name="osb")
                        for ofc in range(2):
                            if qb == NQB - 1:
                                # attention is drained by now: reuse the st
                                # pool for projection double-buffering and
                                # the idle scalar engine for evacuation
                                prj = stp.tile([128, 512], F32, tag="st",
                                               name="prjt")
                            else:
                                prj = prjp.tile([128, 512], F32, tag="prj",
                                                name="prj")
                            for cc in range(NFB):
                                nc.tensor.matmul(
                                    prj,
                                    yt[:, cc, tt * 128:(tt + 1) * 128],
                                    wp_sb[:, cc, ofc * 512:(ofc + 1) * 512],
                                    start=(cc == 0), stop=(cc == NFB - 1))
                            if qb == NQB - 1:
                                nc.scalar.copy(
                                    osb[:, ofc * 512:(ofc + 1) * 512], prj)
                            else:
                                nc.vector.tensor_copy(
                                    osb[:, ofc * 512:(ofc + 1) * 512], prj)
                        nc.sync.dma_start(
                            out_d[qb * QB + tt * 128:qb * QB + (tt + 1) * 128, :],
                            osb)

    nc.compile()
    return nc


def _host_inputs(x, Wk, Wq, Wv, Wp):
    """Build the 8 per-core input maps (host-side slicing/transposes)."""
    ident_np = np.eye(128, dtype=np.float16)
    p = np.arange(128)[:, None]
    jj = np.arange(128)[None, :]
    tri_np = np.where(jj < p, 0.0, 1.0).astype(np.float16)
    ones_np = np.ones((1, 64), dtype=np.float16)

    in_maps = []
    for c in range(N_CORES):
        b, hg = c // 2, c % 2
        fs = slice(hg * FPC, (hg + 1) * FPC)
        in_maps.append({
            "xT": np.ascontiguousarray(x[b].T).astype(np.float16),
            "wqT": np.ascontiguousarray((Wq[fs, :] * 0.125).T).astype(np.float16),
            "wkT": np.ascontiguousarray(Wk[fs, :].T).astype(np.float16),
            "wvT": np.ascontiguousarray(Wv[fs, :].T).astype(np.float16),
            "wpT": np.ascontiguousarray(Wp[:, fs].T).astype(np.float16),
            "ident": ident_np,
            "tri": tri_np,
            "ones": ones_np,
        })
    return in_maps


def kernel(x, Wk, Wq, Wv, Wp, bp, _trace=False):
    x = np.asarray(x, dtype=np.float32)
    Wk = np.asarray(Wk, dtype=np.float32)
    Wq = np.asarray(Wq, dtype=np.float32)
    Wv = np.asarray(Wv, dtype=np.float32)
    Wp = np.asarray(Wp, dtype=np.float32)
    bp = np.asarray(bp, dtype=np.float32)

    if "nc" not in _cached:
        _cached["nc"] = _build_program()
    nc = _cached["nc"]

    in_maps = _host_inputs(x, Wk, Wq, Wv, Wp)
    res = run_bass_kernel_spmd(nc, in_maps, core_ids=list(range(N_CORES)),
                               trace=_trace)
    _cached["last_result"] = res

    out = np.empty((B, T, C), dtype=np.float32)
    for b in range(B):
        out[b] = (res.results[2 * b]["out"].astype(np.float32)
                  + res.results[2 * b + 1]["out"]
                  + bp[None, :])
    return out



# revision 17
# speedup vs baseline: 1.6355x; 1.6355x over previous
"""Causal self-attention on 8 Trainium2 NeuronCores.

Full inputs in, full output out. Sharding: core c -> (batch b = c//2,
head-group hg = c%2 covering 8 of 16 heads). Each core computes QKV
projections for its head slice, causal flash-attention in a transposed
layout (S^T = keys x queries, so softmax denominators come from a ones
column appended to V and no on-device transposes are needed), and a
partial output projection over its 512 feature columns. The host sums
the two partials per batch and adds the bias.

The emission is a single software pipeline: QKV projection chains for
later token blocks and output-projection chains for earlier query
blocks are interleaved into the attention stream as PE filler, because
attention alone is bounded by the activation engine (exp) rather than
the tensor engine. Softmax denominators are broadcast across the 64
head-dim partitions by the (otherwise idle) GPSIMD engine instead of a
PE matmul. Matmul operands are float16 (full PE rate, fp32 PSUM
accumulation); the two heads of a feature block issue S^T matmuls on
disjoint PE row groups, and diagonal blocks only compute the causally
live trapezoid of columns.
"""
import sys
from collections import deque

if "/opt/trn_rl_repo" not in sys.path:
    sys.path.insert(0, "/opt/trn_rl_repo")

import numpy as np

import concourse.bass as bass
import concourse.tile as tile
from concourse import bacc, mybir
from concourse.bass_utils import run_bass_kernel_spmd

F32 = mybir.dt.float32
F16 = mybir.dt.float16
AF = mybir.ActivationFunctionType

B, T, C = 4, 2048, 1024
H, D = 16, 64
N_CORES = 8
HPC = 8            # heads per core
FPC = HPC * D      # feats per core = 512
QB = 512           # query block
NQB = T // QB      # 4
NKK = T // 128     # 16 key chunks
NCC = C // 128     # 8 contraction chunks
NFB = FPC // 128   # 4 feature blocks (head pairs)

_cached = {}


def _build_program():
    nc = bacc.Bacc("TRN2", target_bir_lowering=False, debug=False,
                   num_devices=N_CORES)

    xT_d = nc.dram_tensor("xT", [C, T], F16, kind="ExternalInput").ap()
    wqT_d = nc.dram_tensor("wqT", [C, FPC], F16, kind="ExternalInput").ap()
    wkT_d = nc.dram_tensor("wkT", [C, FPC], F16, kind="ExternalInput").ap()
    wvT_d = nc.dram_tensor("wvT", [C, FPC], F16, kind="ExternalInput").ap()
    wpT_d = nc.dram_tensor("wpT", [FPC, C], F16, kind="ExternalInput").ap()
    ident_d = nc.dram_tensor("ident", [128, 128], F16, kind="ExternalInput").ap()
    tri_d = nc.dram_tensor("tri", [128, 128], F16, kind="ExternalInput").ap()
    out_d = nc.dram_tensor("out", [T, C], F16, kind="ExternalOutput").ap()

    wq_r = wqT_d.rearrange("(c p) f -> p c f", p=128)
    wk_r = wkT_d.rearrange("(c p) f -> p c f", p=128)
    wv_r = wvT_d.rearrange("(c p) f -> p c f", p=128)
    xT_r = xT_d.rearrange("(c p) t -> p c t", p=128)

    with tile.TileContext(nc) as tc:
        with tc.tile_pool(name="persist", bufs=1) as persist, \
             tc.tile_pool(name="xt", bufs=2) as xtp, \
             tc.tile_pool(name="pt", bufs=18) as ptp, \
             tc.tile_pool(name="yt", bufs=2) as ytp, \
             tc.tile_pool(name="rcpr", bufs=2) as rcpp, \
             tc.tile_pool(name="rcpb", bufs=2) as rbp, \
             tc.tile_pool(name="outsb", bufs=3) as outp, \
             tc.tile_pool(name="st", bufs=2, space="PSUM") as stp, \
             tc.tile_pool(name="pv", bufs=1, space="PSUM") as pvp, \
             tc.tile_pool(name="ch", bufs=2, space="PSUM") as chp:
            qt_sb = persist.tile([128, NFB, T], F16, tag="qt")
            kt_sb = persist.tile([128, NFB, T], F16, tag="kt")
            # V' = [V | 1...1] (64 ones columns): the PV matmul then emits
            # the softmax denominator replicated across partitions 64..127,
            # so no cross-partition broadcast is ever needed
            v_sb = persist.tile([128, NKK, HPC, 2 * D], F16, tag="v")
            ident = persist.tile([128, 128], F16, tag="ident")
            tri = persist.tile([128, 128], F16, tag="tri")
            wq_sb = persist.tile([128, NCC, FPC], F16, tag="wq")
            wk_sb = persist.tile([128, NCC, FPC], F16, tag="wk")
            wv_sb = persist.tile([128, NCC, FPC], F16, tag="wv")
            wp_sb = persist.tile([128, NFB, C], F16, tag="wp")

            # warm-up stationary only needs to exist, not hold real data
            nc.gpsimd.memset(ident, 0.0)
            nc.sync.dma_start(tri, tri_d)
            nc.gpsimd.memset(v_sb[:, :, :, D:2 * D], 1.0)

            xt_tiles = {}

            def emit_x_dma(tb):
                # two half-tile DMAs: fewer SP issue slots (565ns each) than
                # per-cc chunks, while the first matmuls can still start
                # after the first half lands
                xt = xtp.tile([128, NCC, QB], F16, tag="xt", name=f"xt{tb}")
                for h in range(2):
                    cs = slice(4 * h, 4 * h + 4)
                    nc.sync.dma_start(xt[:, cs, :],
                                      xT_r[:, cs, tb * QB:(tb + 1) * QB])
                xt_tiles[tb] = xt

            # first Q chain needs xt h1 + wq h1 — land those two first
            xt0 = xtp.tile([128, NCC, QB], F16, tag="xt", name="xt0")
            xt_tiles[0] = xt0
            h1, h2 = slice(0, 4), slice(4, 8)
            nc.sync.dma_start(xt0[:, h1, :], xT_r[:, h1, 0:QB])
            nc.sync.dma_start(wq_sb[:, h1, :], wq_r[:, h1, :])
            nc.sync.dma_start(xt0[:, h2, :], xT_r[:, h2, 0:QB])
            nc.sync.dma_start(wq_sb[:, h2, :], wq_r[:, h2, :])
            for w_sb, w_r in ((wk_sb, wk_r), (wv_sb, wv_r)):
                for cs in (h1, h2):
                    nc.sync.dma_start(w_sb[:, cs, :], w_r[:, cs, :])

            # warm the PE clock gate while input DMAs stream in
            warm = chp.tile([128, 512], F32, tag="ch", name="warm")
            for _ in range(60):
                nc.tensor.matmul(warm[:, 0:128], ident, ident,
                                 start=True, stop=True, skip_group_check=True)

            # ---------------- pipeline chunk emitters ----------------
            def qk_chunk(tb, fb, which):
                w_sb = wq_sb if which == "q" else wk_sb
                dst = qt_sb if which == "q" else kt_sb
                ps = chp.tile([128, 512], F32, tag="ch", name=f"{which}ps")
                xt = xt_tiles[tb]
                for cc in range(NCC):
                    nc.tensor.matmul(
                        ps, w_sb[:, cc, fb * 128:(fb + 1) * 128], xt[:, cc, :],
                        start=(cc == 0), stop=(cc == NCC - 1))
                nc.vector.tensor_copy(dst[:, fb, tb * QB:(tb + 1) * QB], ps)

            def v_chunk(tb, tt):
                ps = chp.tile([128, 512], F32, tag="ch", name="vps")
                xt = xt_tiles[tb]
                for cc in range(NCC):
                    nc.tensor.matmul(
                        ps, xt[:, cc, tt * 128:(tt + 1) * 128], wv_sb[:, cc, :],
                        start=(cc == 0), stop=(cc == NCC - 1))
                nc.vector.tensor_copy(
                    v_sb[:, tb * 4 + tt, :, 0:D],
                    ps.rearrange("p (h d) -> p h d", h=HPC))

            osb_tiles = {}

            def proj_chunk(qb, tt, ofc, yt_tile):
                prj = chp.tile([128, 512], F32, tag="ch", name="prj")
                for cc in range(NFB):
                    nc.tensor.matmul(
                        prj, yt_tile[:, cc, tt * 128:(tt + 1) * 128],
                        wp_sb[:, cc, ofc * 512:(ofc + 1) * 512],
                        start=(cc == 0), stop=(cc == NFB - 1))
                if ofc == 0:
                    osb_tiles[(qb, tt)] = outp.tile([128, C], F16, tag="osb",
                                                    name="osb")
                osb = osb_tiles[(qb, tt)]
                # per-half output DMA shortens the end-of-kernel drain; the
                # very last block evacuates in quarters for the same reason
                nsp = 2 if (qb, tt, ofc) == (NQB - 1, 3, 1) else 1
                for sp in range(nsp):
                    w = 512 // nsp
                    c0 = ofc * 512 + sp * w
                    nc.vector.tensor_copy(osb[:, c0:c0 + w],
                                          prj[:, sp * w:(sp + 1) * w])
                    nc.sync.dma_start(
                        out_d[qb * QB + tt * 128:qb * QB + (tt + 1) * 128,
                              c0:c0 + w],
                        osb[:, c0:c0 + w])

            # pending PE filler: list of dicts(kind, tb, fb/tt, est_ns, fn)
            pending = deque()

            def drain(budget_ns):
                while budget_ns > 0 and pending:
                    e = pending.popleft()
                    e["fn"]()
                    budget_ns -= e["est"]

            def force(pred):
                nonlocal pending
                run, keep = [], []
                for e in pending:
                    (run if pred(e) else keep).append(e)
                pending = deque(keep)
                for e in run:
                    e["fn"]()
                return sum(e["est"] for e in run)

            def queue_qkv(tb):
                for fb in range(NFB):
                    for which in ("q", "k"):
                        pending.append(dict(
                            kind=which, tb=tb, fb=fb, est=1750,
                            fn=(lambda tb=tb, fb=fb, w=which:
                                qk_chunk(tb, fb, w))))
                for tt in range(4):
                    pending.append(dict(
                        kind="v", tb=tb, tt=tt, est=1750,
                        fn=(lambda tb=tb, tt=tt: v_chunk(tb, tt))))

            tri_b = bass.AP(tri.tensor, tri.offset,
                            [tri.ap[0], [0, 2], tri.ap[1]])

            def attention(qb):
                nkk = 4 * qb + 4
                yt_tile = ytp.tile([128, NFB, QB], F16, tag="yt", name="yt")
                for fb in range(NFB):
                    forced = force(lambda e: e["kind"] in ("q", "k")
                                   and e["tb"] <= qb and e["fb"] == fb)
                    ptiles = []
                    for kk in range(nkk):
                        dl = kk - 4 * qb
                        j0 = 128 * dl if dl >= 0 else 0
                        st = stp.tile([128, 2, QB], F32, tag="st", name="st")
                        for h2 in range(2):
                            p0, p1 = 64 * h2, 64 * h2 + 64
                            nc.tensor.matmul(
                                st[:, h2, j0:QB],
                                kt_sb[p0:p1, fb, kk * 128:(kk + 1) * 128],
                                qt_sb[p0:p1, fb, qb * QB + j0:(qb + 1) * QB],
                                start=True, stop=True, skip_group_check=True)
                        pt = ptp.tile([128, 2, QB], F16, tag="pt", name="pt")
                        nc.scalar.activation(
                            pt[:, :, j0:QB], st[:, :, j0:QB], AF.Exp)
                        if dl >= 0:
                            # zero the causally-dead triangle of the diagonal
                            # band (both heads in one strided op; the 0-step
                            # middle dim re-reads the same mask tile)
                            band = pt[:, :, j0:j0 + 128]
                            nc.vector.tensor_mul(band, band, tri_b)
                        ptiles.append(pt)
                    forced += force(lambda e: e["kind"] == "v"
                                    and e["tb"] <= qb)
                    budget = max(0, max(350 * nkk, 2600) - forced)
                    if qb == NQB - 1:
                        # hold back filler to cover the final normalize
                        # chain's latency before proj(3) can start
                        avail = sum(e["est"] for e in pending) - 3500
                        budget = max(0, min(budget, avail))
                    drain(budget)
                    pv = [pvp.tile([128, QB], F32, tag=f"pv{h2}",
                                   name=f"pv{h2}") for h2 in range(2)]
                    for kk in range(nkk):
                        dl = kk - 4 * qb
                        j0 = 128 * dl if dl >= 0 else 0
                        for h2 in range(2):
                            nc.tensor.matmul(
                                pv[h2][:, j0:QB], v_sb[:, kk, 2 * fb + h2, :],
                                ptiles[kk][:, h2, j0:QB],
                                start=(kk == 0), stop=(kk == nkk - 1),
                                skip_group_check=True)
                    for h2 in range(2):
                        p0, p1 = 64 * h2, 64 * h2 + 64
                        # partition-shifting moves only via tensor_copy;
                        # custom-ISA reciprocal stays partition-aligned
                        den = rcpp.tile([64, QB], F32, tag="rcpr", name="den")
                        nc.vector.tensor_copy(den, pv[h2][D:2 * D, :])
                        rb = rbp.tile([64, QB], F32, tag="rcpb", name="rcpb")
                        nc.vector.reciprocal_approx_fast(out=rb, in_=den)
                        nc.vector.tensor_mul(yt_tile[p0:p1, fb, :],
                                             pv[h2][0:D, :], rb)
                return yt_tile

            # ---------------- the pipeline ----------------
            for fb in range(NFB):
                qk_chunk(0, fb, "q")
                qk_chunk(0, fb, "k")
            for tt in range(4):
                v_chunk(0, tt)
            emit_x_dma(1)
            for fb in range(NFB):
                qk_chunk(1, fb, "q")
                qk_chunk(1, fb, "k")
            for tt in range(4):
                v_chunk(1, tt)
            nc.sync.dma_start(wp_sb, wpT_d.rearrange("(c p) f -> p c f", p=128))

            queue_qkv(2)
            queue_qkv(3)

            yts = {}
            for qb in range(NQB):
                if qb + 2 <= 3:
                    emit_x_dma(qb + 2)
                yts[qb] = attention(qb)
                for tt in range(4):
                    for ofc in range(2):
                        pending.append(dict(
                            kind="p", tb=-1, est=1100,
                            fn=(lambda qb=qb, tt=tt, ofc=ofc:
                                proj_chunk(qb, tt, ofc, yts[qb]))))

            force(lambda e: True)

    nc.compile()
    return nc


def _host_inputs(x, Wk, Wq, Wv, Wp):
    """Build the 8 per-core input maps (host-side slicing/transposes)."""
    ident_np = np.eye(128, dtype=np.float16)
    p = np.arange(128)[:, None]
    jj = np.arange(128)[None, :]
    tri_np = np.where(jj < p, 0.0, 1.0).astype(np.float16)

    in_maps = []
    for c in range(N_CORES):
        b, hg = c // 2, c % 2
        fs = slice(hg * FPC, (hg + 1) * FPC)
        in_maps.append({
            "xT": np.ascontiguousarray(x[b].T).astype(np.float16),
            "wqT": np.ascontiguousarray((Wq[fs, :] * 0.125).T).astype(np.float16),
            "wkT": np.ascontiguousarray(Wk[fs, :].T).astype(np.float16),
            "wvT": np.ascontiguousarray(Wv[fs, :].T).astype(np.float16),
            "wpT": np.ascontiguousarray(Wp[:, fs].T).astype(np.float16),
            "ident": ident_np,
            "tri": tri_np,
        })
    return in_maps


def kernel(x, Wk, Wq, Wv, Wp, bp, _trace=False):
    x = np.asarray(x, dtype=np.float32)
    Wk = np.asarray(Wk, dtype=np.float32)
    Wq = np.asarray(Wq, dtype=np.float32)
    Wv = np.asarray(Wv, dtype=np.float32)
    Wp = np.asarray(Wp, dtype=np.float32)
    bp = np.asarray(bp, dtype=np.float32)

    if "nc" not in _cached:
        _cached["nc"] = _build_program()
    nc = _cached["nc"]

    in_maps = _host_inputs(x, Wk, Wq, Wv, Wp)
    res = run_bass_kernel_spmd(nc, in_maps, core_ids=list(range(N_CORES)),
                               trace=_trace)
    _cached["last_result"] = res

    out = np.empty((B, T, C), dtype=np.float32)
    for b in range(B):
        out[b] = (res.results[2 * b]["out"].astype(np.float32)
                  + res.results[2 * b + 1]["out"].astype(np.float32)
                  + bp[None, :])
    return out
